# revision 27
# baseline (speedup 1.0000x reference)
"""Trainium2 Bass kernel for the DAN classifier (gather + segment-mean + MLP + BCE).

Data-parallel across 8 NeuronCores: each core owns 512 whole sentences
(segments). Host splits the token stream at sentence boundaries, buckets each
core's tokens by (segment-group of 128, vocab-quarter of 25600) so embedding
row indices fit in int16 for the bulk dma_gather, and pads each bucket to a
common capacity. On device: dma_gather (4 parallel SWDGE queues, one per
vocab quarter - each queue's descriptors are generated by a different pair of
Q7 cores) pulls bf16 embedding rows from HBM; a one-hot(segment) matmul on
the TensorEngine accumulates per-segment sums in fp32 PSUM; the tiny MLP
head + BCE run on-chip; each core emits its partial loss. Host sums the 8
partials (the all-reduce of the scalar loss).
"""

import sys

try:
    import concourse  # noqa: F401
except ImportError:
    sys.path.insert(0, "/opt/trn_rl_repo")

import ml_dtypes
import numpy as np

import concourse.tile as tile
from concourse import bacc, mybir
from concourse.bass_utils import run_bass_kernel_spmd

# Problem constants (hardcoded per harness contract).
V = 100000
H = 128
B = 4096
T = 409600
N_CORES = 8

SEGS_PER_CORE = B // N_CORES          # 512
# Tapered segment groups: early groups big (pipeline fill), last group small
# so the final async descriptor-generation lag is short.
GROUP_SEGS = (128, 128, 128, 96, 32)
GROUP_STARTS = (0, 128, 256, 384, 480)
N_GROUPS = len(GROUP_SEGS)
N_QUARTERS = 4                        # vocab quarters
VQ = 25600                            # vocab rows per quarter (int16-safe)

F32 = mybir.dt.float32
BF16 = mybir.dt.bfloat16
I16 = mybir.dt.int16
BF16_NP = ml_dtypes.bfloat16


def _build(nc, c_sub, tiles_sb, ranges):
    """Emit the SPMD per-core graph. c_sub = padded tokens per sub-block."""
    n_tiles = c_sub // 128  # token tiles of 128 per sub-block
    n_sub = N_GROUPS * N_QUARTERS

    # ---- DRAM parameters (per-core shards arrive via in_maps) ----
    embed = nc.dram_tensor("embed", [V, H], BF16, kind="ExternalInput")
    idx_d = nc.dram_tensor("idx", [128, n_sub * (c_sub // 16)], I16,
                           kind="ExternalInput")
    seg_d = nc.dram_tensor("seg", [128, n_sub * n_tiles], BF16,
                           kind="ExternalInput")
    recip_d = nc.dram_tensor("recip", [128, N_GROUPS * 128], F32,
                             kind="ExternalInput")
    iota_d = nc.dram_tensor("iota", [128, 128], BF16, kind="ExternalInput")
    y_d = nc.dram_tensor("y", [1, SEGS_PER_CORE], F32, kind="ExternalInput")
    w_hid_d = nc.dram_tensor("w_hid", [H, H], F32, kind="ExternalInput")
    b_hid_d = nc.dram_tensor("b_hid", [H, 1], F32, kind="ExternalInput")
    w_out_d = nc.dram_tensor("w_out", [H, 1], F32, kind="ExternalInput")
    b_out_d = nc.dram_tensor("b_out", [1, 1], F32, kind="ExternalInput")
    out_d = nc.dram_tensor("out", [1, 1], F32, kind="ExternalOutput")

    with tile.TileContext(nc) as tc:
        with (
            tc.tile_pool(name="const", bufs=1) as cpool,
            tc.tile_pool(name="gather", bufs=3) as gpool,
            tc.tile_pool(name="onehot", bufs=2 * N_QUARTERS) as opool,
            tc.tile_pool(name="mlp", bufs=1) as mpool,
            tc.tile_pool(name="psum", bufs=2, space="PSUM") as ppool,
            tc.tile_pool(name="psum_mlp", bufs=1, space="PSUM") as pmpool,
        ):
            # ---- warm activation tables (3-slot table cache: tanh/exp/ln
            # stay resident for the whole per-group chain) ----
            warm = cpool.tile([1, 1], F32)
            for fn in (mybir.ActivationFunctionType.Tanh,
                       mybir.ActivationFunctionType.Exp,
                       mybir.ActivationFunctionType.Ln):
                nc.scalar.activation(out=warm[:], in_=warm[:], func=fn)

            # ---- constants / metadata loads ----
            idx_sb = cpool.tile([128, n_sub * (c_sub // 16)], I16)
            seg_sb = cpool.tile([128, n_sub * n_tiles], BF16)
            gq = N_QUARTERS * (c_sub // 16)
            gs = N_QUARTERS * n_tiles
            for g in range(N_GROUPS):
                nc.sync.dma_start(out=idx_sb[:, g * gq : (g + 1) * gq],
                                  in_=idx_d[:, g * gq : (g + 1) * gq])
                nc.sync.dma_start(out=seg_sb[:, g * gs : (g + 1) * gs],
                                  in_=seg_d[:, g * gs : (g + 1) * gs])
            iota_sb = cpool.tile([128, 128], BF16)
            nc.sync.dma_start(out=iota_sb[:], in_=iota_d[:])
            recip_sb = cpool.tile([128, N_GROUPS * 128], F32)
            nc.sync.dma_start(out=recip_sb[:], in_=recip_d[:])
            y_sb = cpool.tile([1, SEGS_PER_CORE], F32)
            nc.sync.dma_start(out=y_sb[:], in_=y_d[:])
            w_hid_sb = cpool.tile([H, H], F32)
            nc.sync.dma_start(out=w_hid_sb[:], in_=w_hid_d[:])
            b_hid_sb = cpool.tile([H, 1], F32)
            nc.sync.dma_start(out=b_hid_sb[:], in_=b_hid_d[:])
            w_out_sb = cpool.tile([H, 1], F32)
            nc.sync.dma_start(out=w_out_sb[:], in_=w_out_d[:])
            b_out_sb = cpool.tile([1, 1], F32)
            nc.sync.dma_start(out=b_out_sb[:], in_=b_out_d[:])

            sent_t = mpool.tile([128, SEGS_PER_CORE], F32)  # [H, seg]
            psum_hid = pmpool.tile([128, SEGS_PER_CORE], F32, tag="psum_hid")
            hid = mpool.tile([128, SEGS_PER_CORE], F32)
            ep = mpool.tile([1, SEGS_PER_CORE], F32)
            sp = mpool.tile([1, SEGS_PER_CORE], F32)
            sp_sums = mpool.tile([1, N_GROUPS], F32)
            x_sb = mpool.tile([1, SEGS_PER_CORE], F32)
            yx = mpool.tile([1, SEGS_PER_CORE], F32)
            yx_sums = mpool.tile([1, N_GROUPS], F32)

            c16 = c_sub // 16

            def build_onehot(g):
                """Emit is_equal one-hot builds for all 4 quarters of group g."""
                ohs = []
                for q in range(N_QUARTERS):
                    sb = g * N_QUARTERS + q
                    tsb = tiles_sb[sb]
                    oh = opool.tile([128, n_tiles, 128], BF16, tag="onehot")
                    nc.vector.tensor_tensor(
                        out=oh[:, :tsb, :],
                        in0=seg_sb[:, sb * n_tiles : sb * n_tiles + tsb]
                        .rearrange("p (t u) -> p t u", u=1)
                        .to_broadcast([128, tsb, 128]),
                        in1=iota_sb[:]
                        .rearrange("p (u m) -> p u m", u=1)
                        .to_broadcast([128, tsb, 128]),
                        op=mybir.AluOpType.is_equal,
                    )
                    ohs.append(oh)
                return ohs

            # one-hot builds run ahead of the (serialized) descriptor
            # generation so the last group's matmuls fire the moment its
            # gather lands instead of waiting ~10us of IS_EQ on the tail.
            oh_by_group = {0: build_onehot(0)}
            for g in range(N_GROUPS):
                if g + 1 < N_GROUPS:
                    oh_by_group[g + 1] = build_onehot(g + 1)
                gt = gpool.tile([128, N_QUARTERS, n_tiles, 128], BF16,
                                tag="gather")
                # flipped one-hot matmul: the gathered tile is the WEIGHTS
                # (lhsT [slots, H]) and the one-hot the rhs, so the output is
                # sent.T [H, segs] (no transpose step) and, because slots are
                # seg-sorted, each tile only streams its ~10-col segment
                # range instead of all 128. First matmul of the group streams
                # the full 128 cols with start=True to zero-init the psum.
                psum_s = ppool.tile([128, 128], F32, tag="psum_s")
                ohs = oh_by_group.pop(g)
                first = True
                for q in (1, 2, 3, 0):
                    sb = g * N_QUARTERS + q
                    qrows = min(VQ, V - q * VQ)
                    tsb = tiles_sb[sb]
                    oh = ohs[q]
                    rngs = ranges[sb]
                    # single gather per (group, quarter): each extra gather
                    # instruction costs ~1.16us of serialized Q7 time. Only
                    # the very last gather gets a 2-tile sliver so the final
                    # drain+matmul tail is short.
                    cut = tsb
                    if g == N_GROUPS - 1 and q == 0 and tsb > 2:
                        cut = tsb - 2
                    for (lo, hi) in (((0, cut), (cut, tsb)) if cut < tsb
                                     else ((0, tsb),)):
                        nidx = (hi - lo) * 128
                        nc.gpsimd.dma_gather(
                            gt[:, q, lo:hi, :],
                            embed[q * VQ : q * VQ + qrows, :],
                            idx_sb[:, sb * c16 + lo * 8 :
                                   sb * c16 + lo * 8 + nidx // 16],
                            nidx,
                            nidx,
                            H,
                            single_packet=False,
                            queue_num=q,
                        )
                        for j in range(lo, hi):
                            last = q == 0 and j == tsb - 1
                            if first:
                                nc.tensor.matmul(
                                    psum_s[:],
                                    lhsT=gt[:, q, j, :],
                                    rhs=oh[:, j, :],
                                    start=True,
                                    stop=last,
                                )
                                first = False
                            else:
                                a, b = rngs[j]
                                nc.tensor.matmul(
                                    psum_s[:, a:b],
                                    lhsT=gt[:, q, j, :],
                                    rhs=oh[:, j, a:b],
                                    start=False,
                                    stop=last,
                                )

                # segment means for this group: sent.T = psum.T * (1/count),
                # written straight into sent_t (already [H, seg])
                gstart, gsize = GROUP_STARTS[g], GROUP_SEGS[g]
                nc.vector.tensor_tensor(
                    out=sent_t[:, gstart : gstart + gsize],
                    in0=psum_s[:, :gsize],
                    in1=recip_sb[:, g * 128 : g * 128 + gsize],
                    op=mybir.AluOpType.mult,
                )
                nc.tensor.matmul(psum_hid[:, gstart : gstart + gsize],
                                 lhsT=w_hid_sb[:],
                                 rhs=sent_t[:, gstart : gstart + gsize],
                                 start=True, stop=True)
                nc.scalar.activation(
                    out=hid[:, gstart : gstart + gsize],
                    in_=psum_hid[:, gstart : gstart + gsize],
                    func=mybir.ActivationFunctionType.Tanh,
                    bias=b_hid_sb[:, 0:1],
                )
                # per-group W_out matmul (own psum tile) + bias-add into the
                # SBUF staging row: overlaps the next gather; the 512-wide
                # exp/ln/BCE runs once at the tail on SBUF data.
                sl = slice(gstart, gstart + gsize)
                psum_pg = ppool.tile([1, 128], F32, tag="psum_p")
                nc.tensor.matmul(psum_pg[:, :gsize], lhsT=w_out_sb[:],
                                 rhs=hid[:, sl], start=True, stop=True)
                nc.vector.tensor_scalar(
                    out=x_sb[:, sl], in0=psum_pg[:, :gsize],
                    scalar1=b_out_sb[0:1, 0:1],
                    scalar2=None, op0=mybir.AluOpType.add,
                )

            # ---- BCE on x = logit (tail): sum(ln(1+e^x)) - sum(y*x) ----
            sp_sum = mpool.tile([1, 1], F32)
            nc.scalar.activation(
                out=ep[:], in_=x_sb[:],
                func=mybir.ActivationFunctionType.Exp,
            )
            nc.scalar.activation(
                out=sp[:], in_=ep[:],
                func=mybir.ActivationFunctionType.Ln,
                bias=1.0, accum_out=sp_sum[:],
            )
            nc.vector.tensor_tensor(out=yx[:], in0=y_sb[:], in1=x_sb[:],
                                    op=mybir.AluOpType.mult)
            yx_sum = mpool.tile([1, 1], F32)
            nc.vector.tensor_reduce(out=yx_sum[:], in_=yx[:],
                                    axis=mybir.AxisListType.X,
                                    op=mybir.AluOpType.add)
            loss = mpool.tile([1, 1], F32)
            nc.vector.tensor_tensor(out=loss[:], in0=sp_sum[:], in1=yx_sum[:],
                                    op=mybir.AluOpType.subtract)
            nc.sync.dma_start(out=out_d[:], in_=loss[:])

    nc.compile()
    return nc


def _prep_inputs(token_ids, segment_ids, y_true, embed_table, W_hid, b_hid,
                 W_out, b_out):
    """Host-side shard + bucket + pad. Returns (c_sub, in_maps)."""
    token_ids = np.asarray(token_ids, dtype=np.int64)
    segment_ids = np.asarray(segment_ids, dtype=np.int64)
    y_true = np.asarray(y_true, dtype=np.float32)
    embed_bf16 = np.ascontiguousarray(
        np.asarray(embed_table, dtype=np.float32).astype(BF16_NP))

    # sentence-aligned core boundaries
    bounds = np.searchsorted(segment_ids, np.arange(0, B + 1, SEGS_PER_CORE))
    counts = np.bincount(segment_ids, minlength=B).astype(np.float32)
    recip_all = 1.0 / np.maximum(counts, 1.0)

    # bucket tokens per (core, group, quarter)
    per_core = []
    c_max = 0
    for c in range(N_CORES):
        lo, hi = bounds[c], bounds[c + 1]
        tid = token_ids[lo:hi]
        seg_loc = segment_ids[lo:hi] - c * SEGS_PER_CORE
        starts = np.asarray(GROUP_STARTS, dtype=np.int64)
        grp = np.searchsorted(starts[1:], seg_loc, side="right")
        seg_in_grp = (seg_loc - starts[grp]).astype(np.float32)
        q = tid // VQ
        loc_idx = (tid - q * VQ).astype(np.int64)
        subs = []
        for g in range(N_GROUPS):
            for qq in range(N_QUARTERS):
                sel = (grp == g) & (q == qq)
                li, sg = loc_idx[sel], seg_in_grp[sel]
                # seg-major order (id-minor for HBM locality): a 128-slot
                # tile then spans only a handful of segments, so the flipped
                # matmul streams ~10 one-hot columns instead of 128.
                order = np.lexsort((li, sg))
                subs.append((li[order], sg[order]))
                c_max = max(c_max, int(sel.sum()))
        per_core.append(subs)

    c_sub = ((c_max + 127) // 128) * 128
    n_tiles = c_sub // 128
    n_sub = N_GROUPS * N_QUARTERS
    sb_max = [0] * n_sub
    for c in range(N_CORES):
        for sbi, (li, sg) in enumerate(per_core[c]):
            sb_max[sbi] = max(sb_max[sbi], li.shape[0])
    tiles_sb = tuple((m + 127) // 128 for m in sb_max)

    # per-(bucket, tile) segment col range, unioned across cores (SPMD needs
    # one compile-time range); pad slots (seg -1) excluded.
    ranges = []
    for sbi in range(n_sub):
        tsb = tiles_sb[sbi]
        lo = [128] * tsb
        hi = [0] * tsb
        for c in range(N_CORES):
            sg = per_core[c][sbi][1]
            for j in range(tsb):
                part = sg[j * 128 : (j + 1) * 128]
                if part.size:
                    lo[j] = min(lo[j], int(part.min()))
                    hi[j] = max(hi[j], int(part.max()) + 1)
        ranges.append(tuple((min(a, b), max(a + 1, b))
                            for a, b in zip(lo, hi)))
    ranges = tuple(ranges)

    iota = np.broadcast_to(np.arange(128, dtype=np.float32),
                           (128, 128)).astype(BF16_NP)
    in_maps = []
    for c in range(N_CORES):
        idx_arr = np.zeros((128, n_sub * (c_sub // 16)), dtype=np.int16)
        seg_arr = np.full((128, n_sub * n_tiles), -1.0, dtype=BF16_NP)
        for sbi, (li, sg) in enumerate(per_core[c]):
            n = li.shape[0]
            ip = np.zeros(c_sub, dtype=np.int16)
            ip[:n] = li
            sp = np.full(c_sub, -1.0, dtype=np.float32)
            sp[:n] = sg
            wrapped = ip.reshape(c_sub // 16, 16).T  # [16, c_sub//16]
            idx_arr[:, sbi * (c_sub // 16) : (sbi + 1) * (c_sub // 16)] = (
                np.tile(wrapped, (8, 1))
            )
            seg_arr[:, sbi * n_tiles : (sbi + 1) * n_tiles] = (
                sp.reshape(n_tiles, 128).T.astype(BF16_NP)
            )
        # recip broadcast to all 128 partitions: sent.T[h, s] scale is along
        # the free (segment) dim in the flipped layout.
        recip_c = np.ones((128, N_GROUPS * 128), dtype=np.float32)
        for g in range(N_GROUPS):
            gstart, gsize = GROUP_STARTS[g], GROUP_SEGS[g]
            recip_c[:, g * 128 : g * 128 + gsize] = recip_all[
                c * SEGS_PER_CORE + gstart : c * SEGS_PER_CORE + gstart + gsize
            ][None, :]
        in_maps.append({
            "embed": embed_bf16,
            "idx": idx_arr,
            "seg": seg_arr,
            "recip": recip_c,
            "iota": iota,
            "y": np.ascontiguousarray(
                y_true[c * SEGS_PER_CORE : (c + 1) * SEGS_PER_CORE]
            ).reshape(1, SEGS_PER_CORE),
            "w_hid": np.ascontiguousarray(np.asarray(W_hid, dtype=np.float32)),
            "b_hid": np.asarray(b_hid, dtype=np.float32).reshape(H, 1),
            "w_out": np.ascontiguousarray(np.asarray(W_out, dtype=np.float32)),
            "b_out": np.asarray(b_out, dtype=np.float32).reshape(1, 1),
        })
    return c_sub, tiles_sb, ranges, in_maps


_CACHE = {}


def _get_nc(c_sub, tiles_sb, ranges):
    key = (c_sub, tiles_sb, ranges)
    nc = _CACHE.get(key)
    if nc is None:
        nc = bacc.Bacc("TRN2", target_bir_lowering=False, debug=False,
                       num_devices=N_CORES, num_swdge_queues=N_QUARTERS)
        _build(nc, c_sub, tiles_sb, ranges)
        _CACHE[key] = nc
    return nc


def kernel(token_ids, segment_ids, y_true, embed_table, W_hid, b_hid, W_out,
           b_out, _trace=False, _trace_kwargs=None):
    c_sub, tiles_sb, ranges, in_maps = _prep_inputs(token_ids, segment_ids,
                                                    y_true, embed_table,
                                                    W_hid, b_hid, W_out,
                                                    b_out)
    nc = _get_nc(c_sub, tiles_sb, ranges)
    res = run_bass_kernel_spmd(nc, in_maps, core_ids=list(range(N_CORES)),
                               trace=_trace, **(_trace_kwargs or {}))
    total = np.float64(0.0)
    for r in res.results:
        total += np.float64(r["out"][0, 0])
    out = np.array(np.float32(total))
    if _trace:
        return out, res
    return out



# revision 31
# speedup vs baseline: 1.0495x; 1.0495x over previous
"""Trainium2 Bass kernel for the DAN classifier (gather + segment-mean + MLP + BCE).

Data-parallel across 8 NeuronCores: each core owns 512 whole sentences
(segments). Host splits the token stream at sentence boundaries, buckets each
core's tokens by (segment-group of 128, vocab-quarter of 25600) so embedding
row indices fit in int16 for the bulk dma_gather, and pads each bucket to a
common capacity. On device: dma_gather (4 parallel SWDGE queues, one per
vocab quarter - each queue's descriptors are generated by a different pair of
Q7 cores) pulls bf16 embedding rows from HBM; a one-hot(segment) matmul on
the TensorEngine accumulates per-segment sums in fp32 PSUM; the tiny MLP
head + BCE run on-chip; each core emits its partial loss. Host sums the 8
partials (the all-reduce of the scalar loss).
"""

import sys

try:
    import concourse  # noqa: F401
except ImportError:
    sys.path.insert(0, "/opt/trn_rl_repo")

import ml_dtypes
import numpy as np

import concourse.tile as tile
from concourse import bacc, mybir
from concourse.bass_utils import run_bass_kernel_spmd

# Problem constants (hardcoded per harness contract).
V = 100000
H = 128
B = 4096
T = 409600
N_CORES = 8

SEGS_PER_CORE = B // N_CORES          # 512
# Tapered segment groups: early groups big (pipeline fill), last group small
# so the final async descriptor-generation lag is short.
GROUP_SEGS = (128, 128, 128, 96, 32)
GROUP_STARTS = (0, 128, 256, 384, 480)
N_GROUPS = len(GROUP_SEGS)
N_QUARTERS = 4                        # vocab quarters
VQ = 25600                            # vocab rows per quarter (int16-safe)

F32 = mybir.dt.float32
BF16 = mybir.dt.bfloat16
I16 = mybir.dt.int16
BF16_NP = ml_dtypes.bfloat16


def _build(nc, c_sub, tiles_sb, ranges):
    """Emit the SPMD per-core graph. c_sub = padded tokens per sub-block."""
    n_tiles = c_sub // 128  # token tiles of 128 per sub-block
    n_sub = N_GROUPS * N_QUARTERS

    # ---- DRAM parameters (per-core shards arrive via in_maps) ----
    embed = nc.dram_tensor("embed", [V, H], BF16, kind="ExternalInput")
    idx_d = nc.dram_tensor("idx", [128, n_sub * (c_sub // 16)], I16,
                           kind="ExternalInput")
    seg_d = nc.dram_tensor("seg", [128, n_sub * n_tiles], BF16,
                           kind="ExternalInput")
    recip_d = nc.dram_tensor("recip", [128, N_GROUPS * 128], F32,
                             kind="ExternalInput")
    iota_d = nc.dram_tensor("iota", [128, 128], BF16, kind="ExternalInput")
    y_d = nc.dram_tensor("y", [1, SEGS_PER_CORE], F32, kind="ExternalInput")
    w_hid_d = nc.dram_tensor("w_hid", [H, H], F32, kind="ExternalInput")
    b_hid_d = nc.dram_tensor("b_hid", [H, 1], F32, kind="ExternalInput")
    w_out_d = nc.dram_tensor("w_out", [H, 1], F32, kind="ExternalInput")
    b_out_d = nc.dram_tensor("b_out", [1, 1], F32, kind="ExternalInput")
    out_d = nc.dram_tensor("out", [1, 1], F32, kind="ExternalOutput")

    with tile.TileContext(nc) as tc:
        with (
            tc.tile_pool(name="const", bufs=1) as cpool,
            tc.tile_pool(name="gather", bufs=3) as gpool,
            tc.tile_pool(name="onehot", bufs=2 * N_QUARTERS) as opool,
            tc.tile_pool(name="mlp", bufs=1) as mpool,
            tc.tile_pool(name="psum", bufs=2, space="PSUM") as ppool,
            tc.tile_pool(name="psum_mlp", bufs=1, space="PSUM") as pmpool,
        ):
            # ---- warm activation tables (3-slot table cache: tanh/exp/ln
            # stay resident for the whole per-group chain) ----
            warm = cpool.tile([1, 1], F32)
            for fn in (mybir.ActivationFunctionType.Tanh,
                       mybir.ActivationFunctionType.Exp,
                       mybir.ActivationFunctionType.Ln):
                nc.scalar.activation(out=warm[:], in_=warm[:], func=fn)

            # ---- constants / metadata loads ----
            idx_sb = cpool.tile([128, n_sub * (c_sub // 16)], I16)
            seg_sb = cpool.tile([128, n_sub * n_tiles], BF16)
            gq = N_QUARTERS * (c_sub // 16)
            gs = N_QUARTERS * n_tiles
            for g in range(N_GROUPS):
                nc.sync.dma_start(out=idx_sb[:, g * gq : (g + 1) * gq],
                                  in_=idx_d[:, g * gq : (g + 1) * gq])
                nc.sync.dma_start(out=seg_sb[:, g * gs : (g + 1) * gs],
                                  in_=seg_d[:, g * gs : (g + 1) * gs])
            iota_sb = cpool.tile([128, 128], BF16)
            nc.sync.dma_start(out=iota_sb[:], in_=iota_d[:])
            recip_sb = cpool.tile([128, N_GROUPS * 128], F32)
            nc.sync.dma_start(out=recip_sb[:], in_=recip_d[:])
            y_sb = cpool.tile([1, SEGS_PER_CORE], F32)
            nc.sync.dma_start(out=y_sb[:], in_=y_d[:])
            w_hid_sb = cpool.tile([H, H], F32)
            nc.sync.dma_start(out=w_hid_sb[:], in_=w_hid_d[:])
            b_hid_sb = cpool.tile([H, 1], F32)
            nc.sync.dma_start(out=b_hid_sb[:], in_=b_hid_d[:])
            w_out_sb = cpool.tile([H, 1], F32)
            nc.sync.dma_start(out=w_out_sb[:], in_=w_out_d[:])
            b_out_sb = cpool.tile([1, 1], F32)
            nc.sync.dma_start(out=b_out_sb[:], in_=b_out_d[:])

            sent_t = mpool.tile([128, SEGS_PER_CORE], F32)  # [H, seg]
            psum_hid = pmpool.tile([128, SEGS_PER_CORE], F32, tag="psum_hid")
            hid = mpool.tile([128, SEGS_PER_CORE], F32)
            psum_p = pmpool.tile([1, SEGS_PER_CORE], F32, tag="psum_p")
            ep = mpool.tile([1, SEGS_PER_CORE], F32)
            sp = mpool.tile([1, SEGS_PER_CORE], F32)
            sp_sums = mpool.tile([1, N_GROUPS], F32)
            x_sb = mpool.tile([1, SEGS_PER_CORE], F32)
            yx = mpool.tile([1, SEGS_PER_CORE], F32)
            yx_sums = mpool.tile([1, N_GROUPS], F32)

            c16 = c_sub // 16

            def build_onehot(g):
                """Emit is_equal one-hot builds for all 4 quarters of group g."""
                ohs = []
                for q in range(N_QUARTERS):
                    sb = g * N_QUARTERS + q
                    tsb = tiles_sb[sb]
                    oh = opool.tile([128, n_tiles, 128], BF16, tag="onehot")
                    nc.vector.tensor_tensor(
                        out=oh[:, :tsb, :],
                        in0=seg_sb[:, sb * n_tiles : sb * n_tiles + tsb]
                        .rearrange("p (t u) -> p t u", u=1)
                        .to_broadcast([128, tsb, 128]),
                        in1=iota_sb[:]
                        .rearrange("p (u m) -> p u m", u=1)
                        .to_broadcast([128, tsb, 128]),
                        op=mybir.AluOpType.is_equal,
                    )
                    ohs.append(oh)
                return ohs

            # one-hot builds run ahead of the (serialized) descriptor
            # generation so the last group's matmuls fire the moment its
            # gather lands instead of waiting ~10us of IS_EQ on the tail.
            oh_by_group = {0: build_onehot(0)}
            for g in range(N_GROUPS):
                if g + 1 < N_GROUPS:
                    oh_by_group[g + 1] = build_onehot(g + 1)
                gt = gpool.tile([128, N_QUARTERS, n_tiles, 128], BF16,
                                tag="gather")
                # flipped one-hot matmul: the gathered tile is the WEIGHTS
                # (lhsT [slots, H]) and the one-hot the rhs, so the output is
                # sent.T [H, segs] (no transpose step) and, because slots are
                # seg-sorted, each tile only streams its ~10-col segment
                # range instead of all 128. First matmul of the group streams
                # the full 128 cols with start=True to zero-init the psum.
                psum_s = ppool.tile([128, 128], F32, tag="psum_s")
                ohs = oh_by_group.pop(g)
                first = True
                for q in (1, 2, 3, 0):
                    sb = g * N_QUARTERS + q
                    qrows = min(VQ, V - q * VQ)
                    tsb = tiles_sb[sb]
                    oh = ohs[q]
                    rngs = ranges[sb]
                    # single gather per (group, quarter): each extra gather
                    # instruction costs ~1.16us of serialized Q7 time. Only
                    # the very last gather gets a 2-tile sliver so the final
                    # drain+matmul tail is short.
                    cut = tsb
                    if g == N_GROUPS - 1 and q == 0 and tsb > 2:
                        cut = tsb - 2
                    for (lo, hi) in (((0, cut), (cut, tsb)) if cut < tsb
                                     else ((0, tsb),)):
                        nidx = (hi - lo) * 128
                        nc.gpsimd.dma_gather(
                            gt[:, q, lo:hi, :],
                            embed[q * VQ : q * VQ + qrows, :],
                            idx_sb[:, sb * c16 + lo * 8 :
                                   sb * c16 + lo * 8 + nidx // 16],
                            nidx,
                            nidx,
                            H,
                            single_packet=False,
                            queue_num=q,
                        )
                        for j in range(lo, hi):
                            last = q == 0 and j == tsb - 1
                            if first:
                                nc.tensor.matmul(
                                    psum_s[:],
                                    lhsT=gt[:, q, j, :],
                                    rhs=oh[:, j, :],
                                    start=True,
                                    stop=last,
                                )
                                first = False
                            else:
                                a, b = rngs[j]
                                nc.tensor.matmul(
                                    psum_s[:, a:b],
                                    lhsT=gt[:, q, j, :],
                                    rhs=oh[:, j, a:b],
                                    start=False,
                                    stop=last,
                                )

                # segment means for this group: sent.T = psum.T * (1/count),
                # written straight into sent_t (already [H, seg])
                gstart, gsize = GROUP_STARTS[g], GROUP_SEGS[g]
                nc.vector.tensor_tensor(
                    out=sent_t[:, gstart : gstart + gsize],
                    in0=psum_s[:, :gsize],
                    in1=recip_sb[:, g * 128 : g * 128 + gsize],
                    op=mybir.AluOpType.mult,
                )
                nc.tensor.matmul(psum_hid[:, gstart : gstart + gsize],
                                 lhsT=w_hid_sb[:],
                                 rhs=sent_t[:, gstart : gstart + gsize],
                                 start=True, stop=True)
                nc.scalar.activation(
                    out=hid[:, gstart : gstart + gsize],
                    in_=psum_hid[:, gstart : gstart + gsize],
                    func=mybir.ActivationFunctionType.Tanh,
                    bias=b_hid_sb[:, 0:1],
                )
            # ---- MLP head (tail) ----
            nc.tensor.matmul(psum_p[:], lhsT=w_out_sb[:], rhs=hid[:],
                             start=True, stop=True)
            sp_sum = mpool.tile([1, 1], F32)
            nc.scalar.activation(
                out=ep[:], in_=psum_p[:],
                func=mybir.ActivationFunctionType.Exp,
                bias=b_out_sb[0:1, 0:1],
            )
            nc.scalar.activation(
                out=sp[:], in_=ep[:],
                func=mybir.ActivationFunctionType.Ln,
                bias=1.0, accum_out=sp_sum[:],
            )
            nc.vector.tensor_scalar(
                out=x_sb[:], in0=psum_p[:], scalar1=b_out_sb[0:1, 0:1],
                scalar2=None, op0=mybir.AluOpType.add,
            )
            nc.vector.tensor_tensor(out=yx[:], in0=y_sb[:], in1=x_sb[:],
                                    op=mybir.AluOpType.mult)
            yx_sum = mpool.tile([1, 1], F32)
            nc.vector.tensor_reduce(out=yx_sum[:], in_=yx[:],
                                    axis=mybir.AxisListType.X,
                                    op=mybir.AluOpType.add)
            loss = mpool.tile([1, 1], F32)
            nc.vector.tensor_tensor(out=loss[:], in0=sp_sum[:], in1=yx_sum[:],
                                    op=mybir.AluOpType.subtract)
            nc.sync.dma_start(out=out_d[:], in_=loss[:])

    nc.compile()
    return nc


def _prep_inputs(token_ids, segment_ids, y_true, embed_table, W_hid, b_hid,
                 W_out, b_out):
    """Host-side shard + bucket + pad. Returns (c_sub, in_maps)."""
    token_ids = np.asarray(token_ids, dtype=np.int64)
    segment_ids = np.asarray(segment_ids, dtype=np.int64)
    y_true = np.asarray(y_true, dtype=np.float32)
    embed_bf16 = np.ascontiguousarray(
        np.asarray(embed_table, dtype=np.float32).astype(BF16_NP))

    # sentence-aligned core boundaries
    bounds = np.searchsorted(segment_ids, np.arange(0, B + 1, SEGS_PER_CORE))
    counts = np.bincount(segment_ids, minlength=B).astype(np.float32)
    recip_all = 1.0 / np.maximum(counts, 1.0)

    # bucket tokens per (core, group, quarter)
    per_core = []
    c_max = 0
    for c in range(N_CORES):
        lo, hi = bounds[c], bounds[c + 1]
        tid = token_ids[lo:hi]
        seg_loc = segment_ids[lo:hi] - c * SEGS_PER_CORE
        starts = np.asarray(GROUP_STARTS, dtype=np.int64)
        grp = np.searchsorted(starts[1:], seg_loc, side="right")
        seg_in_grp = (seg_loc - starts[grp]).astype(np.float32)
        q = tid // VQ
        loc_idx = (tid - q * VQ).astype(np.int64)
        subs = []
        for g in range(N_GROUPS):
            for qq in range(N_QUARTERS):
                sel = (grp == g) & (q == qq)
                li, sg = loc_idx[sel], seg_in_grp[sel]
                # seg-major order (id-minor for HBM locality): a 128-slot
                # tile then spans only a handful of segments, so the flipped
                # matmul streams ~10 one-hot columns instead of 128.
                order = np.lexsort((li, sg))
                subs.append((li[order], sg[order]))
                c_max = max(c_max, int(sel.sum()))
        per_core.append(subs)

    c_sub = ((c_max + 127) // 128) * 128
    n_tiles = c_sub // 128
    n_sub = N_GROUPS * N_QUARTERS
    sb_max = [0] * n_sub
    for c in range(N_CORES):
        for sbi, (li, sg) in enumerate(per_core[c]):
            sb_max[sbi] = max(sb_max[sbi], li.shape[0])
    tiles_sb = tuple((m + 127) // 128 for m in sb_max)

    # per-(bucket, tile) segment col range, unioned across cores (SPMD needs
    # one compile-time range); pad slots (seg -1) excluded.
    ranges = []
    for sbi in range(n_sub):
        tsb = tiles_sb[sbi]
        lo = [128] * tsb
        hi = [0] * tsb
        for c in range(N_CORES):
            sg = per_core[c][sbi][1]
            for j in range(tsb):
                part = sg[j * 128 : (j + 1) * 128]
                if part.size:
                    lo[j] = min(lo[j], int(part.min()))
                    hi[j] = max(hi[j], int(part.max()) + 1)
        ranges.append(tuple((min(a, b), max(a + 1, b))
                            for a, b in zip(lo, hi)))
    ranges = tuple(ranges)

    iota = np.broadcast_to(np.arange(128, dtype=np.float32),
                           (128, 128)).astype(BF16_NP)
    in_maps = []
    for c in range(N_CORES):
        idx_arr = np.zeros((128, n_sub * (c_sub // 16)), dtype=np.int16)
        seg_arr = np.full((128, n_sub * n_tiles), -1.0, dtype=BF16_NP)
        for sbi, (li, sg) in enumerate(per_core[c]):
            n = li.shape[0]
            ip = np.zeros(c_sub, dtype=np.int16)
            ip[:n] = li
            sp = np.full(c_sub, -1.0, dtype=np.float32)
            sp[:n] = sg
            wrapped = ip.reshape(c_sub // 16, 16).T  # [16, c_sub//16]
            idx_arr[:, sbi * (c_sub // 16) : (sbi + 1) * (c_sub // 16)] = (
                np.tile(wrapped, (8, 1))
            )
            seg_arr[:, sbi * n_tiles : (sbi + 1) * n_tiles] = (
                sp.reshape(n_tiles, 128).T.astype(BF16_NP)
            )
        # recip broadcast to all 128 partitions: sent.T[h, s] scale is along
        # the free (segment) dim in the flipped layout.
        recip_c = np.ones((128, N_GROUPS * 128), dtype=np.float32)
        for g in range(N_GROUPS):
            gstart, gsize = GROUP_STARTS[g], GROUP_SEGS[g]
            recip_c[:, g * 128 : g * 128 + gsize] = recip_all[
                c * SEGS_PER_CORE + gstart : c * SEGS_PER_CORE + gstart + gsize
            ][None, :]
        in_maps.append({
            "embed": embed_bf16,
            "idx": idx_arr,
            "seg": seg_arr,
            "recip": recip_c,
            "iota": iota,
            "y": np.ascontiguousarray(
                y_true[c * SEGS_PER_CORE : (c + 1) * SEGS_PER_CORE]
            ).reshape(1, SEGS_PER_CORE),
            "w_hid": np.ascontiguousarray(np.asarray(W_hid, dtype=np.float32)),
            "b_hid": np.asarray(b_hid, dtype=np.float32).reshape(H, 1),
            "w_out": np.ascontiguousarray(np.asarray(W_out, dtype=np.float32)),
            "b_out": np.asarray(b_out, dtype=np.float32).reshape(1, 1),
        })
    return c_sub, tiles_sb, ranges, in_maps


_CACHE = {}


def _get_nc(c_sub, tiles_sb, ranges):
    key = (c_sub, tiles_sb, ranges)
    nc = _CACHE.get(key)
    if nc is None:
        nc = bacc.Bacc("TRN2", target_bir_lowering=False, debug=False,
                       num_devices=N_CORES, num_swdge_queues=N_QUARTERS)
        _build(nc, c_sub, tiles_sb, ranges)
        _CACHE[key] = nc
    return nc


def kernel(token_ids, segment_ids, y_true, embed_table, W_hid, b_hid, W_out,
           b_out, _trace=False, _trace_kwargs=None):
    c_sub, tiles_sb, ranges, in_maps = _prep_inputs(token_ids, segment_ids,
                                                    y_true, embed_table,
                                                    W_hid, b_hid, W_out,
                                                    b_out)
    nc = _get_nc(c_sub, tiles_sb, ranges)
    res = run_bass_kernel_spmd(nc, in_maps, core_ids=list(range(N_CORES)),
                               trace=_trace, **(_trace_kwargs or {}))
    total = np.float64(0.0)
    for r in res.results:
        total += np.float64(r["out"][0, 0])
    out = np.array(np.float32(total))
    if _trace:
        return out, res
    return out



# revision 39
# speedup vs baseline: 1.0532x; 1.0036x over previous
"""Trainium2 Bass kernel for the DAN classifier (gather + segment-mean + MLP + BCE).

Data-parallel across 8 NeuronCores: each core owns 512 whole sentences
(segments). Host splits the token stream at sentence boundaries, buckets each
core's tokens by (segment-group of 128, vocab-quarter of 25600) so embedding
row indices fit in int16 for the bulk dma_gather, and pads each bucket to a
common capacity. On device: dma_gather (4 parallel SWDGE queues, one per
vocab quarter - each queue's descriptors are generated by a different pair of
Q7 cores) pulls bf16 embedding rows from HBM; a one-hot(segment) matmul on
the TensorEngine accumulates per-segment sums in fp32 PSUM; the tiny MLP
head + BCE run on-chip; each core emits its partial loss. Host sums the 8
partials (the all-reduce of the scalar loss).
"""

import sys

try:
    import concourse  # noqa: F401
except ImportError:
    sys.path.insert(0, "/opt/trn_rl_repo")

import ml_dtypes
import numpy as np

import concourse.tile as tile
from concourse import bacc, mybir
from concourse.bass_utils import run_bass_kernel_spmd

# Problem constants (hardcoded per harness contract).
V = 100000
H = 128
B = 4096
T = 409600
N_CORES = 8

SEGS_PER_CORE = B // N_CORES          # 512
# Tapered segment groups: early groups big (pipeline fill), last group small
# so the final async descriptor-generation lag is short.
GROUP_SEGS = (128, 128, 128, 96, 32)
GROUP_STARTS = (0, 128, 256, 384, 480)
N_GROUPS = len(GROUP_SEGS)
N_QUARTERS = 4                        # vocab quarters
VQ = 25600                            # vocab rows per quarter (int16-safe)

F32 = mybir.dt.float32
BF16 = mybir.dt.bfloat16
I16 = mybir.dt.int16
BF16_NP = ml_dtypes.bfloat16


def _build(nc, c_sub, tiles_sb, ranges):
    """Emit the SPMD per-core graph. c_sub = padded tokens per sub-block."""
    n_tiles = c_sub // 128  # token tiles of 128 per sub-block
    n_sub = N_GROUPS * N_QUARTERS

    # ---- DRAM parameters (per-core shards arrive via in_maps) ----
    embed = nc.dram_tensor("embed", [V, H], BF16, kind="ExternalInput")
    idx_d = nc.dram_tensor("idx", [128, n_sub * (c_sub // 16)], I16,
                           kind="ExternalInput")
    seg_d = nc.dram_tensor("seg", [128, n_sub * n_tiles], BF16,
                           kind="ExternalInput")
    recip_d = nc.dram_tensor("recip", [128, N_GROUPS * 128], F32,
                             kind="ExternalInput")
    iota_d = nc.dram_tensor("iota", [128, 128], BF16, kind="ExternalInput")
    y_d = nc.dram_tensor("y", [128, SEGS_PER_CORE // 128], F32,
                         kind="ExternalInput")
    w_hid_d = nc.dram_tensor("w_hid", [H, H], F32, kind="ExternalInput")
    b_hid_d = nc.dram_tensor("b_hid", [H, 1], F32, kind="ExternalInput")
    w_out_d = nc.dram_tensor("w_out", [H, 1], F32, kind="ExternalInput")
    b_out_d = nc.dram_tensor("b_out", [128, 1], F32, kind="ExternalInput")
    out_d = nc.dram_tensor("out", [1, 1], F32, kind="ExternalOutput")

    with tile.TileContext(nc) as tc:
        with (
            tc.tile_pool(name="const", bufs=1) as cpool,
            tc.tile_pool(name="gather", bufs=3) as gpool,
            tc.tile_pool(name="onehot", bufs=2 * N_QUARTERS) as opool,
            tc.tile_pool(name="mlp", bufs=1) as mpool,
            tc.tile_pool(name="psum", bufs=2, space="PSUM") as ppool,
            tc.tile_pool(name="psum_mlp", bufs=1, space="PSUM") as pmpool,
        ):
            # ---- warm activation tables ----
            warm = cpool.tile([1, 1], F32)
            for fn in (mybir.ActivationFunctionType.Exp,
                       mybir.ActivationFunctionType.Ln,
                       mybir.ActivationFunctionType.Tanh):
                nc.scalar.activation(out=warm[:], in_=warm[:], func=fn)

            # ---- constants / metadata loads ----
            idx_sb = cpool.tile([128, n_sub * (c_sub // 16)], I16)
            seg_sb = cpool.tile([128, n_sub * n_tiles], BF16)
            gq = N_QUARTERS * (c_sub // 16)
            gs = N_QUARTERS * n_tiles
            for g in range(N_GROUPS):
                nc.sync.dma_start(out=idx_sb[:, g * gq : (g + 1) * gq],
                                  in_=idx_d[:, g * gq : (g + 1) * gq])
                nc.sync.dma_start(out=seg_sb[:, g * gs : (g + 1) * gs],
                                  in_=seg_d[:, g * gs : (g + 1) * gs])
            iota_sb = cpool.tile([128, 128], BF16)
            nc.sync.dma_start(out=iota_sb[:], in_=iota_d[:])
            recip_sb = cpool.tile([128, N_GROUPS * 128], F32)
            nc.sync.dma_start(out=recip_sb[:], in_=recip_d[:])
            n_bk = SEGS_PER_CORE // 128  # 4 segment blocks of 128
            y_sb = cpool.tile([128, n_bk], F32)
            nc.sync.dma_start(out=y_sb[:], in_=y_d[:])
            w_hid_sb = cpool.tile([H, H], F32)
            nc.sync.dma_start(out=w_hid_sb[:], in_=w_hid_d[:])
            b_hid_sb = cpool.tile([H, 1], F32)
            nc.sync.dma_start(out=b_hid_sb[:], in_=b_hid_d[:])
            w_out_sb = cpool.tile([H, 1], F32)
            nc.sync.dma_start(out=w_out_sb[:], in_=w_out_d[:])
            b_out_sb = cpool.tile([128, 1], F32)
            nc.sync.dma_start(out=b_out_sb[:], in_=b_out_d[:])
            ones_sb = cpool.tile([128, 1], F32)
            nc.vector.memset(ones_sb[:], 1.0)

            sent_t = mpool.tile([128, SEGS_PER_CORE], F32)  # [H, seg]
            psum_hid = pmpool.tile([128, SEGS_PER_CORE], F32, tag="psum_hid")
            hid = mpool.tile([128, SEGS_PER_CORE], F32)

            c16 = c_sub // 16

            def build_onehot(g):
                """Emit is_equal one-hot builds for all 4 quarters of group g."""
                ohs = []
                for q in range(N_QUARTERS):
                    sb = g * N_QUARTERS + q
                    tsb = tiles_sb[sb]
                    oh = opool.tile([128, n_tiles, 128], BF16, tag="onehot")
                    nc.vector.tensor_tensor(
                        out=oh[:, :tsb, :],
                        in0=seg_sb[:, sb * n_tiles : sb * n_tiles + tsb]
                        .rearrange("p (t u) -> p t u", u=1)
                        .to_broadcast([128, tsb, 128]),
                        in1=iota_sb[:]
                        .rearrange("p (u m) -> p u m", u=1)
                        .to_broadcast([128, tsb, 128]),
                        op=mybir.AluOpType.is_equal,
                    )
                    ohs.append(oh)
                return ohs

            # one-hot builds run ahead of the (serialized) descriptor
            # generation so the last group's matmuls fire the moment its
            # gather lands instead of waiting ~10us of IS_EQ on the tail.
            oh_by_group = {0: build_onehot(0)}
            for g in range(N_GROUPS):
                if g + 1 < N_GROUPS:
                    oh_by_group[g + 1] = build_onehot(g + 1)
                gt = gpool.tile([128, N_QUARTERS, n_tiles, 128], BF16,
                                tag="gather")
                # flipped one-hot matmul: the gathered tile is the WEIGHTS
                # (lhsT [slots, H]) and the one-hot the rhs, so the output is
                # sent.T [H, segs] (no transpose step) and, because slots are
                # seg-sorted, each tile only streams its ~10-col segment
                # range instead of all 128. First matmul of the group streams
                # the full 128 cols with start=True to zero-init the psum.
                psum_s = ppool.tile([128, 128], F32, tag="psum_s")
                ohs = oh_by_group.pop(g)
                first = True
                for q in (1, 2, 3, 0):
                    sb = g * N_QUARTERS + q
                    qrows = min(VQ, V - q * VQ)
                    tsb = tiles_sb[sb]
                    oh = ohs[q]
                    rngs = ranges[sb]
                    # single gather per (group, quarter): each extra gather
                    # instruction costs ~1.16us of serialized Q7 time. Only
                    # the very last gather gets a 2-tile sliver so the final
                    # drain+matmul tail is short.
                    cut = tsb
                    if g == N_GROUPS - 1 and q == 0 and tsb > 2:
                        cut = tsb - 2
                    for (lo, hi) in (((0, cut), (cut, tsb)) if cut < tsb
                                     else ((0, tsb),)):
                        nidx = (hi - lo) * 128
                        nc.gpsimd.dma_gather(
                            gt[:, q, lo:hi, :],
                            embed[q * VQ : q * VQ + qrows, :],
                            idx_sb[:, sb * c16 + lo * 8 :
                                   sb * c16 + lo * 8 + nidx // 16],
                            nidx,
                            nidx,
                            H,
                            single_packet=False,
                            queue_num=q,
                        )
                        for j in range(lo, hi):
                            last = q == 0 and j == tsb - 1
                            if first:
                                nc.tensor.matmul(
                                    psum_s[:],
                                    lhsT=gt[:, q, j, :],
                                    rhs=oh[:, j, :],
                                    start=True,
                                    stop=last,
                                )
                                first = False
                            else:
                                a, b = rngs[j]
                                nc.tensor.matmul(
                                    psum_s[:, a:b],
                                    lhsT=gt[:, q, j, :],
                                    rhs=oh[:, j, a:b],
                                    start=False,
                                    stop=last,
                                )

                # segment means for this group: sent.T = psum.T * (1/count),
                # written straight into sent_t (already [H, seg])
                gstart, gsize = GROUP_STARTS[g], GROUP_SEGS[g]
                nc.vector.tensor_tensor(
                    out=sent_t[:, gstart : gstart + gsize],
                    in0=psum_s[:, :gsize],
                    in1=recip_sb[:, g * 128 : g * 128 + gsize],
                    op=mybir.AluOpType.mult,
                )
                nc.tensor.matmul(psum_hid[:, gstart : gstart + gsize],
                                 lhsT=w_hid_sb[:],
                                 rhs=sent_t[:, gstart : gstart + gsize],
                                 start=True, stop=True)
                nc.scalar.activation(
                    out=hid[:, gstart : gstart + gsize],
                    in_=psum_hid[:, gstart : gstart + gsize],
                    func=mybir.ActivationFunctionType.Tanh,
                    bias=b_hid_sb[:, 0:1],
                )
                if g == N_GROUPS - 2:
                    # touch exp/ln during the last group's gather so the BCE
                    # tail doesn't pay an ACT_TABLE_LOAD on the critical path
                    for fn in (mybir.ActivationFunctionType.Exp,
                               mybir.ActivationFunctionType.Ln):
                        nc.scalar.activation(out=warm[:], in_=warm[:],
                                             func=fn)
            # ---- MLP head + BCE (tail), laid out [128, 4] so every op is
            # 128-partition-parallel instead of 512 elements on one lane ----
            psum_x = pmpool.tile([128, n_bk], F32, tag="psum_x")
            for k in range(n_bk):
                nc.tensor.matmul(psum_x[:, k : k + 1],
                                 lhsT=hid[:, k * 128 : (k + 1) * 128],
                                 rhs=w_out_sb[:], start=True, stop=True)
            x128 = mpool.tile([128, n_bk], F32)
            nc.vector.tensor_scalar(
                out=x128[:], in0=psum_x[:], scalar1=b_out_sb[:, 0:1],
                scalar2=None, op0=mybir.AluOpType.add,
            )
            # loss terms: softplus(x) - y*x, softplus via exp then ln(1+.)
            ep128 = mpool.tile([128, n_bk], F32)
            nc.scalar.activation(
                out=ep128[:], in_=x128[:],
                func=mybir.ActivationFunctionType.Exp,
            )
            sp128 = mpool.tile([128, n_bk], F32)
            nc.scalar.activation(
                out=sp128[:], in_=ep128[:],
                func=mybir.ActivationFunctionType.Ln,
                bias=1.0,
            )
            yx128 = mpool.tile([128, n_bk], F32)
            nc.vector.tensor_tensor(out=yx128[:], in0=y_sb[:], in1=x128[:],
                                    op=mybir.AluOpType.mult)
            diff = mpool.tile([128, n_bk], F32)
            nc.vector.tensor_tensor(out=diff[:], in0=sp128[:], in1=yx128[:],
                                    op=mybir.AluOpType.subtract)
            red = mpool.tile([128, 1], F32)
            nc.vector.tensor_reduce(out=red[:], in_=diff[:],
                                    axis=mybir.AxisListType.X,
                                    op=mybir.AluOpType.add)
            # partition-sum via ones-matmul: loss = sum_p red[p]
            psum_l = pmpool.tile([1, 1], F32, tag="psum_l")
            nc.tensor.matmul(psum_l[:], lhsT=red[:], rhs=ones_sb[:],
                             start=True, stop=True)
            loss = mpool.tile([1, 1], F32)
            nc.vector.tensor_copy(out=loss[:], in_=psum_l[:])
            nc.sync.dma_start(out=out_d[:], in_=loss[:])

    nc.compile()
    return nc


def _prep_inputs(token_ids, segment_ids, y_true, embed_table, W_hid, b_hid,
                 W_out, b_out):
    """Host-side shard + bucket + pad. Returns (c_sub, in_maps)."""
    token_ids = np.asarray(token_ids, dtype=np.int64)
    segment_ids = np.asarray(segment_ids, dtype=np.int64)
    y_true = np.asarray(y_true, dtype=np.float32)
    embed_bf16 = np.ascontiguousarray(
        np.asarray(embed_table, dtype=np.float32).astype(BF16_NP))

    # sentence-aligned core boundaries
    bounds = np.searchsorted(segment_ids, np.arange(0, B + 1, SEGS_PER_CORE))
    counts = np.bincount(segment_ids, minlength=B).astype(np.float32)
    recip_all = 1.0 / np.maximum(counts, 1.0)

    # bucket tokens per (core, group, quarter)
    per_core = []
    c_max = 0
    for c in range(N_CORES):
        lo, hi = bounds[c], bounds[c + 1]
        tid = token_ids[lo:hi]
        seg_loc = segment_ids[lo:hi] - c * SEGS_PER_CORE
        starts = np.asarray(GROUP_STARTS, dtype=np.int64)
        grp = np.searchsorted(starts[1:], seg_loc, side="right")
        seg_in_grp = (seg_loc - starts[grp]).astype(np.float32)
        q = tid // VQ
        loc_idx = (tid - q * VQ).astype(np.int64)
        subs = []
        for g in range(N_GROUPS):
            for qq in range(N_QUARTERS):
                sel = (grp == g) & (q == qq)
                li, sg = loc_idx[sel], seg_in_grp[sel]
                # seg-major order (id-minor for HBM locality): a 128-slot
                # tile then spans only a handful of segments, so the flipped
                # matmul streams ~10 one-hot columns instead of 128.
                order = np.lexsort((li, sg))
                subs.append((li[order], sg[order]))
                c_max = max(c_max, int(sel.sum()))
        per_core.append(subs)

    c_sub = ((c_max + 127) // 128) * 128
    n_tiles = c_sub // 128
    n_sub = N_GROUPS * N_QUARTERS
    sb_max = [0] * n_sub
    for c in range(N_CORES):
        for sbi, (li, sg) in enumerate(per_core[c]):
            sb_max[sbi] = max(sb_max[sbi], li.shape[0])
    tiles_sb = tuple((m + 127) // 128 for m in sb_max)

    # per-(bucket, tile) segment col range, unioned across cores (SPMD needs
    # one compile-time range); pad slots (seg -1) excluded.
    ranges = []
    for sbi in range(n_sub):
        tsb = tiles_sb[sbi]
        lo = [128] * tsb
        hi = [0] * tsb
        for c in range(N_CORES):
            sg = per_core[c][sbi][1]
            for j in range(tsb):
                part = sg[j * 128 : (j + 1) * 128]
                if part.size:
                    lo[j] = min(lo[j], int(part.min()))
                    hi[j] = max(hi[j], int(part.max()) + 1)
        ranges.append(tuple((min(a, b), max(a + 1, b))
                            for a, b in zip(lo, hi)))
    ranges = tuple(ranges)

    iota = np.broadcast_to(np.arange(128, dtype=np.float32),
                           (128, 128)).astype(BF16_NP)
    in_maps = []
    for c in range(N_CORES):
        idx_arr = np.zeros((128, n_sub * (c_sub // 16)), dtype=np.int16)
        seg_arr = np.full((128, n_sub * n_tiles), -1.0, dtype=BF16_NP)
        for sbi, (li, sg) in enumerate(per_core[c]):
            n = li.shape[0]
            ip = np.zeros(c_sub, dtype=np.int16)
            ip[:n] = li
            sp = np.full(c_sub, -1.0, dtype=np.float32)
            sp[:n] = sg
            wrapped = ip.reshape(c_sub // 16, 16).T  # [16, c_sub//16]
            idx_arr[:, sbi * (c_sub // 16) : (sbi + 1) * (c_sub // 16)] = (
                np.tile(wrapped, (8, 1))
            )
            seg_arr[:, sbi * n_tiles : (sbi + 1) * n_tiles] = (
                sp.reshape(n_tiles, 128).T.astype(BF16_NP)
            )
        # recip broadcast to all 128 partitions: sent.T[h, s] scale is along
        # the free (segment) dim in the flipped layout.
        recip_c = np.ones((128, N_GROUPS * 128), dtype=np.float32)
        for g in range(N_GROUPS):
            gstart, gsize = GROUP_STARTS[g], GROUP_SEGS[g]
            recip_c[:, g * 128 : g * 128 + gsize] = recip_all[
                c * SEGS_PER_CORE + gstart : c * SEGS_PER_CORE + gstart + gsize
            ][None, :]
        in_maps.append({
            "embed": embed_bf16,
            "idx": idx_arr,
            "seg": seg_arr,
            "recip": recip_c,
            "iota": iota,
            "y": np.ascontiguousarray(
                y_true[c * SEGS_PER_CORE : (c + 1) * SEGS_PER_CORE]
                .reshape(SEGS_PER_CORE // 128, 128).T
            ),
            "w_hid": np.ascontiguousarray(np.asarray(W_hid, dtype=np.float32)),
            "b_hid": np.asarray(b_hid, dtype=np.float32).reshape(H, 1),
            "w_out": np.ascontiguousarray(np.asarray(W_out, dtype=np.float32)),
            "b_out": np.full((128, 1), np.float32(np.asarray(b_out).ravel()[0]),
                             dtype=np.float32),
        })
    return c_sub, tiles_sb, ranges, in_maps


_CACHE = {}


def _get_nc(c_sub, tiles_sb, ranges):
    key = (c_sub, tiles_sb, ranges)
    nc = _CACHE.get(key)
    if nc is None:
        nc = bacc.Bacc("TRN2", target_bir_lowering=False, debug=False,
                       num_devices=N_CORES, num_swdge_queues=N_QUARTERS)
        _build(nc, c_sub, tiles_sb, ranges)
        _CACHE[key] = nc
    return nc


def kernel(token_ids, segment_ids, y_true, embed_table, W_hid, b_hid, W_out,
           b_out, _trace=False, _trace_kwargs=None):
    c_sub, tiles_sb, ranges, in_maps = _prep_inputs(token_ids, segment_ids,
                                                    y_true, embed_table,
                                                    W_hid, b_hid, W_out,
                                                    b_out)
    nc = _get_nc(c_sub, tiles_sb, ranges)
    res = run_bass_kernel_spmd(nc, in_maps, core_ids=list(range(N_CORES)),
                               trace=_trace, **(_trace_kwargs or {}))
    total = np.float64(0.0)
    for r in res.results:
        total += np.float64(r["out"][0, 0])
    out = np.array(np.float32(total))
    if _trace:
        return out, res
    return out

